# revision 11
# baseline (speedup 1.0000x reference)
"""Two-layer GCN (GraphConv norm='both') on 8 Trainium2 NeuronCores.

Design v2 (evolved from the pair-gather baseline, 940us -> target ~300us):

The baseline's L2 span was bound by gather DMA *consumption* (512B pair-rows
per edge over 2 SWDGE queues ~ 38GB/s each), not descriptor generation, and
its gather stream could not start before L1's end because the sub-AllGather
instructions sit ahead of the dma_gather instructions in the in-order gpsimd
queue.

Key restructurings:
  1. Edges are grouped by (dst_tile, src_region) with NREG=4 regions instead
     of (dst_tile, parity).  Each region has its OWN gather table (r rows of
     that region AllGathered core-major), and every region table has
     < 32768 rows, so int16 indices address single 128-f16 rows directly:
     gather payload drops 512B -> 256B per edge (the pair trick existed only
     because int16 could not address all 50000 rows).
  2. 4 SWDGE queues round-robin the gather windows (gen pipelines ~4x).
  3. The one-hot selector matrices S1/S2 (were 40MB of HBM streams) are
     built on-chip by one fused DVE op per chunk:
       S[p, j] = (iota[j] == dl[p]) * sval[p]
     from a tiny [128, nchunk, 2] (dl, sval) f16 side table.
  4. The per-region AllGathers are issued from the *scalar* engine's queue,
     so the gpsimd queue holds only dma_gather instructions and region-0
     gathers start as soon as table 0 lands (~region-0 L1 + AG latency),
     instead of after all of L1.
  5. yE window loads rotate across sync/scalar/vector HWDGE queues.
"""

import bisect

import numpy as np

N_NODES = 50000
N_EDGES = 600000
D = 128
N_CORES = 8
NPC = N_NODES // N_CORES          # 6250 nodes per core
NT = (NPC + 127) // 128           # 49 dst tiles per core
REG_TILES = (6, 10, 15, 18)       # dst tiles per region (sums to NT)
REG_ROWS = (768, 1280, 1920, 2282)  # rows per core per region (sums to NPC)
W = 8                             # chunks per gather window (single-packet cap)
NQ = 4                            # SWDGE queues
MT_BUFS = 40                      # gather window lookahead
BT = 4

NREG = len(REG_TILES)
_REG_LO = tuple(int(v) for v in np.cumsum((0,) + REG_ROWS[:-1]))

_CACHE = {}


def _schedule(sched):
    """Expand the shared (static, max-over-cores) schedule into position
    space.  L1: tile-major chunk runs.  L2: region-major then tile-major
    chunk runs, each region run padded to a W-chunk window boundary so a
    gather window touches exactly one region table."""
    C1 = np.array(sched[0])
    C2 = np.array(sched[1])          # [NREG, NT]
    base1 = np.concatenate([[0], np.cumsum(C1)[:-1]])
    nchunk1 = int(C1.sum())

    bases2 = np.zeros((NREG, NT), dtype=np.int64)
    run_end_w = []
    pos = 0
    for r in range(NREG):
        for t in range(NT):
            bases2[r, t] = pos
            pos += int(C2[r, t])
        pos += (-pos) % W
        run_end_w.append(pos // W)
    nchunk2 = pos
    return dict(C1=C1, C2=C2, base1=base1, nchunk1=nchunk1, bases2=bases2,
                run_end_w=run_end_w, nchunk2=nchunk2, nW2=nchunk2 // W)


def _host_prep(x, src, dst, W1, b1, W2, b2):
    x = np.asarray(x, dtype=np.float32)
    src = np.asarray(src, dtype=np.int64)
    dst = np.asarray(dst, dtype=np.int64)
    W1 = np.asarray(W1, dtype=np.float32)
    W2 = np.asarray(W2, dtype=np.float32)
    b1 = np.asarray(b1, dtype=np.float32)
    b2 = np.asarray(b2, dtype=np.float32)

    deg_out = np.bincount(src, minlength=N_NODES).astype(np.float32)
    deg_in = np.bincount(dst, minlength=N_NODES).astype(np.float32)
    norm_src = np.where(deg_out > 0, 1.0 / np.sqrt(np.maximum(deg_out, 1.0)), 0.0)
    norm_dst = np.where(deg_in > 0, 1.0 / np.sqrt(np.maximum(deg_in, 1.0)), 0.0)
    sval = (norm_src[src] * norm_dst[dst]).astype(np.float32)
    x16 = x.astype(np.float16)

    reg_lo = np.array(_REG_LO)
    reg_rows = np.array(REG_ROWS)

    # --- per-core edge grouping by (dst_tile, src_region) ---
    per_core = []
    cnt1 = np.zeros((N_CORES, NT), dtype=np.int64)
    cnt2 = np.zeros((N_CORES, NREG * NT), dtype=np.int64)
    for k in range(N_CORES):
        m = (dst >= k * NPC) & (dst < (k + 1) * NPC)
        s_k = src[m]
        dl_k = dst[m] - k * NPC
        sv_k = sval[m]
        t_k = dl_k >> 7
        rs_k = s_k % NPC
        reg = np.searchsorted(reg_lo, rs_k, side="right") - 1
        g = reg * NT + t_k           # region-major group id
        order = np.lexsort((s_k, g))
        s_k, dl_k, sv_k, g, t_k = (a[order] for a in (s_k, dl_k, sv_k, g, t_k))
        cnt1[k] = np.bincount(t_k, minlength=NT)
        cnt2[k] = np.bincount(g, minlength=NREG * NT)
        per_core.append((s_k, dl_k, sv_k, g, t_k))

    # --- shared static schedule (max over cores) ---
    C1 = np.maximum.reduce([(cnt1[k] + 127) // 128 for k in range(N_CORES)])
    C1 = np.maximum(C1, 1)
    C2 = np.maximum.reduce([(cnt2[k] + 127) // 128 for k in range(N_CORES)])
    C2 = C2.reshape(NREG, NT)
    sched = (tuple(int(v) for v in C1),
             tuple(tuple(int(v) for v in row) for row in C2))
    S = _schedule(sched)
    nchunk1, nchunk2 = S["nchunk1"], S["nchunk2"]
    meta = (nchunk1, nchunk2, tuple(S["run_end_w"]))

    base1_128 = S["base1"] * 128
    bases2_128 = S["bases2"] * 128

    iota = np.ascontiguousarray(
        np.broadcast_to(np.arange(128, dtype=np.float32), (128, 128)))

    in_maps = []
    for k in range(N_CORES):
        s_k, dl_k, sv_k, g, t_k = per_core[k]
        reg = g // NT

        # L2 slot: rank within (region, tile) group
        grp_counts = np.bincount(g, minlength=NREG * NT)
        grp_start = np.concatenate([[0], np.cumsum(grp_counts)[:-1]])
        rank = np.arange(len(g)) - grp_start[g]
        pos2 = bases2_128[reg, g % NT] + rank

        # gather index: row in the region table (core-major concat)
        ks = s_k // NPC
        rs_k = s_k % NPC
        cat = ks * reg_rows[reg] + (rs_k - reg_lo[reg])
        idx16 = np.zeros(nchunk2 * 128, dtype=np.int16)
        idx16[pos2] = cat.astype(np.int16)
        idx_wrapped = np.tile(idx16.reshape(-1, 16).T, (8, 1))

        dlsv2 = np.zeros((128, nchunk2, 2), dtype=np.float32)
        dlsv2[pos2 % 128, pos2 // 128, 0] = (dl_k & 127).astype(np.float32)
        dlsv2[pos2 % 128, pos2 // 128, 1] = sv_k.astype(np.float32)

        # L1 slot: rank within tile group (g-sort is tile-sorted within
        # region, so re-sort by tile only)
        o1 = np.argsort(t_k, kind="stable")
        t1 = t_k[o1]
        grp_counts1 = np.bincount(t1, minlength=NT)
        grp_start1 = np.concatenate([[0], np.cumsum(grp_counts1)[:-1]])
        rank1 = np.arange(len(t1)) - grp_start1[t1]
        pos1 = base1_128[t1] + rank1

        yE = np.zeros((128, nchunk1, 128), dtype=np.float16)
        yE[pos1 % 128, pos1 // 128, :] = x16[s_k[o1]]
        dlsv1 = np.zeros((128, nchunk1, 2), dtype=np.float32)
        dlsv1[pos1 % 128, pos1 // 128, 0] = (dl_k[o1] & 127).astype(np.float32)
        dlsv1[pos1 % 128, pos1 // 128, 1] = sv_k[o1].astype(np.float32)

        in_maps.append(
            {
                "yE": np.ascontiguousarray(yE.reshape(128, nchunk1 * 128)),
                "dlsv1": np.ascontiguousarray(dlsv1),
                "dlsv2": np.ascontiguousarray(dlsv2),
                "idx_all": idx_wrapped,
                "IOTA": iota,
                "W1f": W1.astype(np.float16),
                "W2f": W2.astype(np.float16),
                "B1bc": np.ascontiguousarray(
                    np.broadcast_to(b1, (128, 128)).astype(np.float32)),
                "B2bc": np.ascontiguousarray(
                    np.broadcast_to(b2, (128, 128)).astype(np.float32)),
            }
        )
    return in_maps, sched, meta


def _build_program(sched, meta):
    import concourse.bacc as bacc
    import concourse.mybir as mybir
    import concourse.tile as tile
    from concourse.library_config import mlp

    S = _schedule(sched)
    C1, C2, base1, nchunk1 = S["C1"], S["C2"], S["base1"], S["nchunk1"]
    bases2, run_end_w, nchunk2 = S["bases2"], S["run_end_w"], S["nchunk2"]
    assert meta == (nchunk1, nchunk2, tuple(run_end_w))

    f16 = mybir.dt.float16
    f32 = mybir.dt.float32
    AF = mybir.ActivationFunctionType
    ALU = mybir.AluOpType

    nc = bacc.Bacc("TRN2", target_bir_lowering=False, debug=False,
                   num_devices=N_CORES, num_swdge_queues=NQ)

    yE_d = nc.dram_tensor("yE", [128, nchunk1 * 128], f16, kind="ExternalInput")
    dlsv1_d = nc.dram_tensor("dlsv1", [128, nchunk1, 2], f32,
                             kind="ExternalInput")
    dlsv2_d = nc.dram_tensor("dlsv2", [128, nchunk2, 2], f32,
                             kind="ExternalInput")
    idx_d = nc.dram_tensor("idx_all", [128, nchunk2 * 8], mybir.dt.int16,
                           kind="ExternalInput")
    iota_d = nc.dram_tensor("IOTA", [128, 128], f32, kind="ExternalInput")
    W1_d = nc.dram_tensor("W1f", [128, 128], f16, kind="ExternalInput")
    W2_d = nc.dram_tensor("W2f", [128, 128], f16, kind="ExternalInput")
    B1_d = nc.dram_tensor("B1bc", [128, 128], f32, kind="ExternalInput")
    B2_d = nc.dram_tensor("B2bc", [128, 128], f32, kind="ExternalInput")

    r_parts = [nc.dram_tensor(f"r{i}", [REG_ROWS[i], D], f16, kind="Internal")
               for i in range(NREG)]
    tabs = [nc.dram_tensor(f"T{j}", [N_CORES * REG_ROWS[j], D], f16,
                           kind="Internal", addr_space="Shared")
            for j in range(NREG)]
    out_d = nc.dram_tensor("out", [NPC, D], f32, kind="ExternalOutput")

    with tile.TileContext(nc) as tc:
        with (
            tc.tile_pool(name="consts", bufs=1) as consts,
            tc.tile_pool(name="l1y", bufs=6) as l1y_pool,
            tc.tile_pool(name="s1b", bufs=4) as s1b_pool,
            tc.tile_pool(name="s2b", bufs=4) as s2b_pool,
            tc.tile_pool(name="mt", bufs=MT_BUFS) as mt_pool,
            tc.tile_pool(name="za", bufs=2 * NT) as za_pool,
            tc.tile_pool(name="hb", bufs=10) as hb_pool,
            tc.tile_pool(name="psz", bufs=4, space="PSUM") as psz_pool,
            tc.tile_pool(name="psw", bufs=4, space="PSUM") as psw_pool,
        ):
            nc.gpsimd.load_library(mlp)

            W1f = consts.tile([128, 128], f16, tag="W1f")
            W2f = consts.tile([128, 128], f16, tag="W2f")
            B1bc = consts.tile([128, 128], f32, tag="B1bc")
            B2bc = consts.tile([128, 128], f32, tag="B2bc")
            iota_t = consts.tile([128, 128], f32, tag="iota")
            idx_all = consts.tile([128, nchunk2 * 8], mybir.dt.int16, tag="idx")
            dlsv1_t = consts.tile([128, nchunk1, 2], f32, tag="dlsv1")
            dlsv2_t = consts.tile([128, nchunk2, 2], f32, tag="dlsv2")
            nc.sync.dma_start(W1f[:], W1_d.ap())
            nc.sync.dma_start(W2f[:], W2_d.ap())
            nc.sync.dma_start(B1bc[:], B1_d.ap())
            nc.sync.dma_start(B2bc[:], B2_d.ap())
            nc.sync.dma_start(iota_t[:], iota_d.ap())
            nc.scalar.dma_start(idx_all[:], idx_d.ap())
            nc.scalar.dma_start(dlsv1_t[:], dlsv1_d.ap())
            nc.scalar.dma_start(dlsv2_t[:], dlsv2_d.ap())

            # ---------- writers (node-major row streams to DRAM) ----------
            def make_writer(dram, t_lo, t_hi, dt):
                nfull = min(t_hi, NPC // 128) - t_lo
                h3 = dram.ap()[0: nfull * 128, :].rearrange(
                    "(a p) d -> p a d", p=128)
                state = {}

                def write(t, produce):
                    tl_ = t - t_lo
                    if tl_ < nfull:
                        g = tl_ - tl_ % BT
                        if tl_ % BT == 0:
                            state["buf"] = hb_pool.tile(
                                [128, BT, 128], dt, tag=f"w{dt}", name="wstage")
                        produce(state["buf"][:, tl_ % BT, :])
                        if tl_ % BT == BT - 1 or tl_ == nfull - 1:
                            n = tl_ - g + 1
                            nc.sync.dma_start(h3[:, g: g + n, :],
                                              state["buf"][:, 0:n, :])
                    else:
                        rows = (t_hi * 128 if t_hi < NT else NPC) - t * 128
                        tl = hb_pool.tile([128, 128], dt, tag=f"rag{dt}",
                                          name="wrag")
                        produce(tl[:])
                        nc.sync.dma_start(
                            dram.ap()[tl_ * 128: tl_ * 128 + rows, :],
                            tl[:rows, :])

                return write

            def build_s(pool, dlsv, c, tag):
                s = pool.tile([128, 128], f16, tag=tag, name=tag)
                nc.vector.tensor_scalar(
                    out=s[:], in0=iota_t[:],
                    scalar1=dlsv[:, c, 0:1], scalar2=dlsv[:, c, 1:2],
                    op0=ALU.is_equal, op1=ALU.mult)
                return s

            # ---------------- L1: z1 = yE.T @ S1, r = relu(z1@W1+b1) --------
            l1_tiles = {}
            l1_engines = (nc.sync, nc.scalar)

            def ensure1(w):
                if w in l1_tiles:
                    return l1_tiles[w]
                cb = w * W
                cw = min(W, nchunk1 - cb)
                yt = l1y_pool.tile([128, cw * 128], f16, tag="yt")
                l1_engines[w % 2].dma_start(
                    yt[:], yE_d.ap()[:, cb * 128:(cb + cw) * 128])
                l1_tiles[w] = yt
                return yt

            reg_end_t = np.cumsum(REG_TILES)
            reg_start_t = [0] + list(reg_end_t[:-1])
            writers = [make_writer(r_parts[i], reg_start_t[i],
                                   int(reg_end_t[i]), f16)
                       for i in range(NREG)]

            def sub_ag(i):
                nc.gpsimd.collective_compute(
                    "AllGather", ALU.bypass,
                    replica_groups=[list(range(N_CORES))],
                    ins=[r_parts[i].ap()], outs=[tabs[i].ap()],
                )

            # gather windows are emitted on the gpsimd queue right after the
            # AllGather that fills their region table, interleaved with L1 in
            # program order, so run j's descriptor generation starts as soon
            # as table j lands instead of after the last AllGather issue
            mt_tiles = {}

            def ensure2(w):
                if w in mt_tiles:
                    return mt_tiles[w]
                cb = w * W
                j = bisect.bisect_right(run_end_w, w)
                mt = mt_pool.tile([128, W, 128], f16, tag="mt")
                nc.gpsimd.dma_gather(
                    mt[:], tabs[j].ap(), idx_all[:, cb * 8:(cb + W) * 8],
                    W * 128, W * 128, 128, queue_num=w % NQ)
                mt_tiles[w] = mt
                return mt

            run_w = [(0 if i == 0 else run_end_w[i - 1], run_end_w[i])
                     for i in range(NREG)]

            for t in range(NT):
                zp = psz_pool.tile([128, 128], f32, tag="zp", name="z1")
                c0, c1 = int(base1[t]), int(base1[t] + C1[t])
                for c in range(c0, c1):
                    yt = ensure1(c // W)
                    o = c % W
                    s1 = build_s(s1b_pool, dlsv1_t, c, "s1")
                    nc.tensor.matmul(zp[:], yt[:, o * 128:(o + 1) * 128],
                                     s1[:], start=(c == c0), stop=(c == c1 - 1))
                z1sb = hb_pool.tile([128, 128], f16, tag="zsb", name="z1sb")
                nc.vector.tensor_copy(out=z1sb[:], in_=zp[:])
                pw = psw_pool.tile([128, 128], f32, tag="pw", name="pw1")
                nc.tensor.matmul(pw[:], z1sb[:], W1f[:])

                def produce_r(dst, pw=pw):
                    rt = hb_pool.tile([128, 128], f16, tag="rt", name="rt")
                    nc.vector.tensor_tensor(rt[:], pw[:], B1bc[:], op=ALU.add)
                    nc.vector.tensor_scalar_max(dst, rt[:], 0.0)

                ri = int(np.searchsorted(reg_end_t, t, side="right"))
                writers[ri](t, produce_r)
                if t == reg_end_t[ri] - 1:
                    sub_ag(ri)
                    for w in range(*run_w[ri]):
                        ensure2(w)

            # ---------------- L2: NREG-pass agg ----------------
            def agg_run(t, clist):
                pa = psz_pool.tile([128, 128], f32, tag="zp", name="z2")
                n = len(clist)
                for i, c in enumerate(clist):
                    mt = ensure2(c // W)
                    o = c % W
                    s2 = build_s(s2b_pool, dlsv2_t, c, "s2")
                    nc.tensor.matmul(
                        pa[:], mt[:, o, :], s2[:],
                        start=(i == 0), stop=(i == n - 1))
                return pa

            wr_out = make_writer(out_d, 0, NT, f32)
            zacc = {}
            for rpass in range(NREG):
                last = rpass == NREG - 1
                for t in range(NT):
                    cl = [int(bases2[rpass, t]) + j
                          for j in range(int(C2[rpass, t]))]
                    if cl:
                        pa = agg_run(t, cl)
                        if t in zacc:
                            znew = hb_pool.tile([128, 128], f16, tag="zsb",
                                                name="zadd")
                            nc.vector.tensor_tensor(znew[:], pa[:], zacc[t][:],
                                                    op=ALU.add)
                        else:
                            znew = za_pool.tile([128, 128], f16, tag="zA",
                                                name="zA")
                            nc.vector.tensor_copy(out=znew[:], in_=pa[:])
                        if not last:
                            if t in zacc:
                                zst = za_pool.tile([128, 128], f16, tag="zA",
                                                   name="zA2")
                                nc.vector.tensor_copy(out=zst[:], in_=znew[:])
                                znew = zst
                        zacc[t] = znew
                    elif last and t not in zacc:
                        znew = za_pool.tile([128, 128], f16, tag="zA", name="zA")
                        nc.vector.memset(znew[:], 0.0)
                        zacc[t] = znew
                    if last:
                        pw2 = psw_pool.tile([128, 128], f32, tag="pw",
                                            name="pw2")
                        nc.tensor.matmul(pw2[:], zacc[t][:], W2f[:])
                        wr_out(t, lambda dst, pw2=pw2: nc.vector.tensor_tensor(
                            dst, pw2[:], B2bc[:], op=ALU.add))

    nc.compile()
    return nc


def kernel(x, src, dst, W1, b1, W2, b2):
    from concourse.bass_utils import run_bass_kernel_spmd

    in_maps, sched, meta = _host_prep(x, src, dst, W1, b1, W2, b2)
    key = (sched, meta)
    if key not in _CACHE:
        _CACHE[key] = _build_program(sched, meta)
    nc = _CACHE[key]
    res = run_bass_kernel_spmd(nc, in_maps, core_ids=list(range(N_CORES)))
    out = np.empty((N_NODES, D), dtype=np.float32)
    for k in range(N_CORES):
        out[k * NPC: (k + 1) * NPC] = res.results[k]["out"]
    return out


# revision 12
# speedup vs baseline: 1.3464x; 1.3464x over previous
"""Two-layer GCN (GraphConv norm='both') on 8 Trainium2 NeuronCores.

Design v2 (evolved from the pair-gather baseline, 940us -> target ~300us):

The baseline's L2 span was bound by gather DMA *consumption* (512B pair-rows
per edge over 2 SWDGE queues ~ 38GB/s each), not descriptor generation, and
its gather stream could not start before L1's end because the sub-AllGather
instructions sit ahead of the dma_gather instructions in the in-order gpsimd
queue.

Key restructurings:
  1. Edges are grouped by (dst_tile, src_region) with NREG=4 regions instead
     of (dst_tile, parity).  Each region has its OWN gather table (r rows of
     that region AllGathered core-major), and every region table has
     < 32768 rows, so int16 indices address single 128-f16 rows directly:
     gather payload drops 512B -> 256B per edge (the pair trick existed only
     because int16 could not address all 50000 rows).
  2. 4 SWDGE queues round-robin the gather windows (gen pipelines ~4x).
  3. The one-hot selector matrices S1/S2 (were 40MB of HBM streams) are
     built on-chip by one fused DVE op per chunk:
       S[p, j] = (iota[j] == dl[p]) * sval[p]
     from a tiny [128, nchunk, 2] (dl, sval) f16 side table.
  4. The per-region AllGathers are issued from the *scalar* engine's queue,
     so the gpsimd queue holds only dma_gather instructions and region-0
     gathers start as soon as table 0 lands (~region-0 L1 + AG latency),
     instead of after all of L1.
  5. yE window loads rotate across sync/scalar/vector HWDGE queues.
"""

import bisect

import numpy as np

N_NODES = 50000
N_EDGES = 600000
D = 128
N_CORES = 8
NPC = N_NODES // N_CORES          # 6250 nodes per core
NT = (NPC + 127) // 128           # 49 dst tiles per core
REG_TILES = (6, 10, 15, 18)       # dst tiles per region (sums to NT)
REG_ROWS = (768, 1280, 1920, 2282)  # rows per core per region (sums to NPC)
W = 8                             # chunks per gather window (single-packet cap)
NQ = 4                            # SWDGE queues
MT_BUFS = 40                      # gather window lookahead
BT = 4

NREG = len(REG_TILES)
_REG_LO = tuple(int(v) for v in np.cumsum((0,) + REG_ROWS[:-1]))

_CACHE = {}


def _schedule(sched):
    """Expand the shared (static, max-over-cores) schedule into position
    space.  L1: tile-major chunk runs.  L2: region-major then tile-major
    chunk runs, each region run padded to a W-chunk window boundary so a
    gather window touches exactly one region table."""
    C1 = np.array(sched[0])
    C2 = np.array(sched[1])          # [NREG, NT]
    base1 = np.concatenate([[0], np.cumsum(C1)[:-1]])
    nchunk1 = int(C1.sum())

    bases2 = np.zeros((NREG, NT), dtype=np.int64)
    run_end_w = []
    pos = 0
    for r in range(NREG):
        for t in range(NT):
            bases2[r, t] = pos
            pos += int(C2[r, t])
        pos += (-pos) % W
        run_end_w.append(pos // W)
    nchunk2 = pos
    return dict(C1=C1, C2=C2, base1=base1, nchunk1=nchunk1, bases2=bases2,
                run_end_w=run_end_w, nchunk2=nchunk2, nW2=nchunk2 // W)


def _host_prep(x, src, dst, W1, b1, W2, b2):
    x = np.asarray(x, dtype=np.float32)
    src = np.asarray(src, dtype=np.int64)
    dst = np.asarray(dst, dtype=np.int64)
    W1 = np.asarray(W1, dtype=np.float32)
    W2 = np.asarray(W2, dtype=np.float32)
    b1 = np.asarray(b1, dtype=np.float32)
    b2 = np.asarray(b2, dtype=np.float32)

    deg_out = np.bincount(src, minlength=N_NODES).astype(np.float32)
    deg_in = np.bincount(dst, minlength=N_NODES).astype(np.float32)
    norm_src = np.where(deg_out > 0, 1.0 / np.sqrt(np.maximum(deg_out, 1.0)), 0.0)
    norm_dst = np.where(deg_in > 0, 1.0 / np.sqrt(np.maximum(deg_in, 1.0)), 0.0)
    x16ns = (x * norm_src[:, None]).astype(np.float16)  # ns[src] folded in
    nsrc_pad = np.zeros(NPC + 128, dtype=np.float32)
    ndst_pad = np.zeros(NPC + 128, dtype=np.float32)

    reg_lo = np.array(_REG_LO)
    reg_rows = np.array(REG_ROWS)

    # --- per-core edge grouping by (dst_tile, src_region) ---
    per_core = []
    cnt1 = np.zeros((N_CORES, NT), dtype=np.int64)
    cnt2 = np.zeros((N_CORES, NREG * NT), dtype=np.int64)
    for k in range(N_CORES):
        m = (dst >= k * NPC) & (dst < (k + 1) * NPC)
        s_k = src[m]
        dl_k = dst[m] - k * NPC
        t_k = dl_k >> 7
        rs_k = s_k % NPC
        reg = np.searchsorted(reg_lo, rs_k, side="right") - 1
        g = reg * NT + t_k           # region-major group id
        order = np.lexsort((s_k, g))
        s_k, dl_k, g, t_k = (a[order] for a in (s_k, dl_k, g, t_k))
        cnt1[k] = np.bincount(t_k, minlength=NT)
        cnt2[k] = np.bincount(g, minlength=NREG * NT)
        per_core.append((s_k, dl_k, g, t_k))

    # --- shared static schedule (max over cores) ---
    C1 = np.maximum.reduce([(cnt1[k] + 127) // 128 for k in range(N_CORES)])
    C1 = np.maximum(C1, 1)
    C2 = np.maximum.reduce([(cnt2[k] + 127) // 128 for k in range(N_CORES)])
    C2 = C2.reshape(NREG, NT)
    sched = (tuple(int(v) for v in C1),
             tuple(tuple(int(v) for v in row) for row in C2))
    S = _schedule(sched)
    nchunk1, nchunk2 = S["nchunk1"], S["nchunk2"]
    meta = (nchunk1, nchunk2, tuple(S["run_end_w"]))

    base1_128 = S["base1"] * 128
    bases2_128 = S["bases2"] * 128

    iota = np.ascontiguousarray(
        np.broadcast_to(np.arange(128, dtype=np.float32), (128, 128)))

    in_maps = []
    for k in range(N_CORES):
        s_k, dl_k, g, t_k = per_core[k]
        reg = g // NT

        # L2 slot: rank within (region, tile) group
        grp_counts = np.bincount(g, minlength=NREG * NT)
        grp_start = np.concatenate([[0], np.cumsum(grp_counts)[:-1]])
        rank = np.arange(len(g)) - grp_start[g]
        pos2 = bases2_128[reg, g % NT] + rank

        # gather index: row in the region table (core-major concat)
        ks = s_k // NPC
        rs_k = s_k % NPC
        cat = ks * reg_rows[reg] + (rs_k - reg_lo[reg])
        idx16 = np.zeros(nchunk2 * 128, dtype=np.int16)
        idx16[pos2] = cat.astype(np.int16)
        idx_wrapped = np.tile(idx16.reshape(-1, 16).T, (8, 1))

        DL2 = np.full((128, nchunk2), -1.0, dtype=np.float32)
        DL2[pos2 % 128, pos2 // 128] = (dl_k & 127).astype(np.float32)

        # L1 slot: rank within tile group (g-sort is tile-sorted within
        # region, so re-sort by tile only)
        o1 = np.argsort(t_k, kind="stable")
        t1 = t_k[o1]
        grp_counts1 = np.bincount(t1, minlength=NT)
        grp_start1 = np.concatenate([[0], np.cumsum(grp_counts1)[:-1]])
        rank1 = np.arange(len(t1)) - grp_start1[t1]
        pos1 = base1_128[t1] + rank1

        yE = np.zeros((128, nchunk1, 128), dtype=np.float16)
        yE[pos1 % 128, pos1 // 128, :] = x16ns[s_k[o1]]
        DL1 = np.full((128, nchunk1), -1.0, dtype=np.float32)
        DL1[pos1 % 128, pos1 // 128] = (dl_k[o1] & 127).astype(np.float32)

        nsrc_pad[:NPC] = norm_src[k * NPC:(k + 1) * NPC]
        ndst_pad[:NPC] = norm_dst[k * NPC:(k + 1) * NPC]
        NSRC = np.ascontiguousarray(
            nsrc_pad[: NT * 128].reshape(NT, 128).T.astype(np.float32))
        NDST = np.ascontiguousarray(
            ndst_pad[: NT * 128].reshape(NT, 128).T.astype(np.float32))

        in_maps.append(
            {
                "yE": np.ascontiguousarray(yE.reshape(128, nchunk1 * 128)),
                "DL1": DL1,
                "DL2": DL2,
                "NSRC": NSRC,
                "NDST": NDST,
                "idx_all": idx_wrapped,
                "IOTA": iota,
                "W1f": W1.astype(np.float16),
                "W2f": W2.astype(np.float16),
                "B1bc": np.ascontiguousarray(
                    np.broadcast_to(b1, (128, 128)).astype(np.float32)),
                "B2bc": np.ascontiguousarray(
                    np.broadcast_to(b2, (128, 128)).astype(np.float32)),
            }
        )
    return in_maps, sched, meta


def _build_program(sched, meta):
    import concourse.bacc as bacc
    import concourse.mybir as mybir
    import concourse.tile as tile
    from concourse.bass import broadcast_tensor_aps
    from concourse.library_config import mlp

    S = _schedule(sched)
    C1, C2, base1, nchunk1 = S["C1"], S["C2"], S["base1"], S["nchunk1"]
    bases2, run_end_w, nchunk2 = S["bases2"], S["run_end_w"], S["nchunk2"]
    assert meta == (nchunk1, nchunk2, tuple(run_end_w))

    f16 = mybir.dt.float16
    f32 = mybir.dt.float32
    AF = mybir.ActivationFunctionType
    ALU = mybir.AluOpType

    nc = bacc.Bacc("TRN2", target_bir_lowering=False, debug=False,
                   num_devices=N_CORES, num_swdge_queues=NQ)

    yE_d = nc.dram_tensor("yE", [128, nchunk1 * 128], f16, kind="ExternalInput")
    dl1_d = nc.dram_tensor("DL1", [128, nchunk1], f32, kind="ExternalInput")
    dl2_d = nc.dram_tensor("DL2", [128, nchunk2], f32, kind="ExternalInput")
    nsrc_d = nc.dram_tensor("NSRC", [128, NT], f32, kind="ExternalInput")
    ndst_d = nc.dram_tensor("NDST", [128, NT], f32, kind="ExternalInput")
    idx_d = nc.dram_tensor("idx_all", [128, nchunk2 * 8], mybir.dt.int16,
                           kind="ExternalInput")
    iota_d = nc.dram_tensor("IOTA", [128, 128], f32, kind="ExternalInput")
    W1_d = nc.dram_tensor("W1f", [128, 128], f16, kind="ExternalInput")
    W2_d = nc.dram_tensor("W2f", [128, 128], f16, kind="ExternalInput")
    B1_d = nc.dram_tensor("B1bc", [128, 128], f32, kind="ExternalInput")
    B2_d = nc.dram_tensor("B2bc", [128, 128], f32, kind="ExternalInput")

    r_parts = [nc.dram_tensor(f"r{i}", [REG_ROWS[i], D], f16, kind="Internal")
               for i in range(NREG)]
    tabs = [nc.dram_tensor(f"T{j}", [N_CORES * REG_ROWS[j], D], f16,
                           kind="Internal", addr_space="Shared")
            for j in range(NREG)]
    out_d = nc.dram_tensor("out", [NPC, D], f32, kind="ExternalOutput")

    with tile.TileContext(nc) as tc:
        with (
            tc.tile_pool(name="consts", bufs=1) as consts,
            tc.tile_pool(name="l1y", bufs=6) as l1y_pool,
            tc.tile_pool(name="s1b", bufs=6) as s1b_pool,
            tc.tile_pool(name="s2b", bufs=6) as s2b_pool,
            tc.tile_pool(name="mt", bufs=MT_BUFS) as mt_pool,
            tc.tile_pool(name="za", bufs=2 * NT) as za_pool,
            tc.tile_pool(name="hb", bufs=10) as hb_pool,
            tc.tile_pool(name="psz", bufs=4, space="PSUM") as psz_pool,
            tc.tile_pool(name="psw", bufs=4, space="PSUM") as psw_pool,
        ):
            nc.gpsimd.load_library(mlp)

            W1f = consts.tile([128, 128], f16, tag="W1f")
            W2f = consts.tile([128, 128], f16, tag="W2f")
            B1bc = consts.tile([128, 128], f32, tag="B1bc")
            B2bc = consts.tile([128, 128], f32, tag="B2bc")
            iota_t = consts.tile([128, 128], f32, tag="iota")
            idx_all = consts.tile([128, nchunk2 * 8], mybir.dt.int16, tag="idx")
            dl1_t = consts.tile([128, nchunk1], f32, tag="dl1")
            dl2_t = consts.tile([128, nchunk2], f32, tag="dl2")
            nsrc_t = consts.tile([128, NT], f32, tag="nsrc")
            ndst_t = consts.tile([128, NT], f32, tag="ndst")
            nc.sync.dma_start(W1f[:], W1_d.ap())
            nc.sync.dma_start(W2f[:], W2_d.ap())
            nc.sync.dma_start(B1bc[:], B1_d.ap())
            nc.sync.dma_start(B2bc[:], B2_d.ap())
            nc.sync.dma_start(iota_t[:], iota_d.ap())
            nc.scalar.dma_start(idx_all[:], idx_d.ap())
            nc.scalar.dma_start(dl1_t[:], dl1_d.ap())
            nc.scalar.dma_start(dl2_t[:], dl2_d.ap())
            nc.scalar.dma_start(nsrc_t[:], nsrc_d.ap())
            nc.scalar.dma_start(ndst_t[:], ndst_d.ap())

            # ---------- writers (node-major row streams to DRAM) ----------
            def make_writer(dram, t_lo, t_hi, dt):
                nfull = min(t_hi, NPC // 128) - t_lo
                h3 = dram.ap()[0: nfull * 128, :].rearrange(
                    "(a p) d -> p a d", p=128)
                state = {}

                def write(t, produce):
                    tl_ = t - t_lo
                    if tl_ < nfull:
                        g = tl_ - tl_ % BT
                        if tl_ % BT == 0:
                            state["buf"] = hb_pool.tile(
                                [128, BT, 128], dt, tag=f"w{dt}", name="wstage")
                        produce(state["buf"][:, tl_ % BT, :])
                        if tl_ % BT == BT - 1 or tl_ == nfull - 1:
                            n = tl_ - g + 1
                            nc.sync.dma_start(h3[:, g: g + n, :],
                                              state["buf"][:, 0:n, :])
                    else:
                        rows = (t_hi * 128 if t_hi < NT else NPC) - t * 128
                        tl = hb_pool.tile([128, 128], dt, tag=f"rag{dt}",
                                          name="wrag")
                        produce(tl[:])
                        nc.sync.dma_start(
                            dram.ap()[tl_ * 128: tl_ * 128 + rows, :],
                            tl[:rows, :])

                return write

            def build_sw(pool, dl_t, cb, cw, tag):
                # one-hot for a whole window in one DVE op:
                # s[p, a, j] = (iota[j] == dl[p, cb+a])
                s = pool.tile([128, cw, 128], f16, tag=tag, name=tag)
                a0 = iota_t[:].rearrange("p (a d) -> p a d", a=1)
                a1 = dl_t[:, cb:cb + cw].rearrange("p (a d) -> p a d", d=1)
                b0, b1 = broadcast_tensor_aps(a0, a1)
                nc.vector.tensor_tensor(s[:], b0, b1, op=ALU.is_equal)
                return s

            # ---------------- L1: z1 = yE.T @ S1, r = relu(z1@W1+b1) --------
            l1_tiles = {}
            l1_engines = (nc.sync, nc.scalar)

            def ensure1(w):
                if w in l1_tiles:
                    return l1_tiles[w]
                cb = w * W
                cw = min(W, nchunk1 - cb)
                yt = l1y_pool.tile([128, cw * 128], f16, tag="yt")
                l1_engines[w % 2].dma_start(
                    yt[:], yE_d.ap()[:, cb * 128:(cb + cw) * 128])
                l1_tiles[w] = yt
                return yt

            reg_end_t = np.cumsum(REG_TILES)
            reg_start_t = [0] + list(reg_end_t[:-1])
            writers = [make_writer(r_parts[i], reg_start_t[i],
                                   int(reg_end_t[i]), f16)
                       for i in range(NREG)]

            def sub_ag(i):
                nc.gpsimd.collective_compute(
                    "AllGather", ALU.bypass,
                    replica_groups=[list(range(N_CORES))],
                    ins=[r_parts[i].ap()], outs=[tabs[i].ap()],
                )

            # gather windows are emitted on the gpsimd queue right after the
            # AllGather that fills their region table, interleaved with L1 in
            # program order, so run j's descriptor generation starts as soon
            # as table j lands instead of after the last AllGather issue
            mt_tiles = {}

            def ensure2(w):
                if w in mt_tiles:
                    return mt_tiles[w]
                cb = w * W
                j = bisect.bisect_right(run_end_w, w)
                mt = mt_pool.tile([128, W, 128], f16, tag="mt")
                nc.gpsimd.dma_gather(
                    mt[:], tabs[j].ap(), idx_all[:, cb * 8:(cb + W) * 8],
                    W * 128, W * 128, 128, queue_num=w % NQ)
                mt_tiles[w] = mt
                return mt

            run_w = [(0 if i == 0 else run_end_w[i - 1], run_end_w[i])
                     for i in range(NREG)]

            s1_tiles = {}

            def ensure_s1(w):
                if w in s1_tiles:
                    return s1_tiles[w]
                cb = w * W
                cw = min(W, nchunk1 - cb)
                s = build_sw(s1b_pool, dl1_t, cb, cw, "s1")
                s1_tiles[w] = s
                return s

            for t in range(NT):
                zp = psz_pool.tile([128, 128], f32, tag="zp", name="z1")
                c0, c1 = int(base1[t]), int(base1[t] + C1[t])
                for c in range(c0, c1):
                    yt = ensure1(c // W)
                    o = c % W
                    s1 = ensure_s1(c // W)
                    nc.tensor.matmul(zp[:], yt[:, o * 128:(o + 1) * 128],
                                     s1[:, o, :], start=(c == c0),
                                     stop=(c == c1 - 1))
                z1sb = hb_pool.tile([128, 128], f16, tag="zsb", name="z1sb")
                nc.scalar.activation(z1sb[:], zp[:], AF.Copy)
                pw = psw_pool.tile([128, 128], f32, tag="pw", name="pw1")
                nc.tensor.matmul(pw[:], z1sb[:], W1f[:])

                def produce_r(dst, pw=pw, t=t):
                    # r = relu((pw * nd + b1) * ns): ns goes into the table
                    # rows (norm_src fold), nd is this layer's dst norm
                    rt = hb_pool.tile([128, 128], f16, tag="rt", name="rt")
                    nc.vector.scalar_tensor_tensor(
                        rt[:], pw[:], ndst_t[:, t:t + 1], B1bc[:],
                        op0=ALU.mult, op1=ALU.add)
                    nc.vector.tensor_scalar(
                        out=dst, in0=rt[:], scalar1=nsrc_t[:, t:t + 1],
                        scalar2=0.0, op0=ALU.mult, op1=ALU.max)

                ri = int(np.searchsorted(reg_end_t, t, side="right"))
                writers[ri](t, produce_r)
                if t == reg_end_t[ri] - 1:
                    sub_ag(ri)
                    for w in range(*run_w[ri]):
                        ensure2(w)

            # ---------------- L2: NREG-pass agg ----------------
            s2_tiles = {}

            def ensure_s2(w):
                if w in s2_tiles:
                    return s2_tiles[w]
                s = build_sw(s2b_pool, dl2_t, w * W, W, "s2")
                s2_tiles[w] = s
                return s

            def agg_run(t, clist):
                pa = psz_pool.tile([128, 128], f32, tag="zp", name="z2")
                n = len(clist)
                for i, c in enumerate(clist):
                    mt = ensure2(c // W)
                    o = c % W
                    s2 = ensure_s2(c // W)
                    nc.tensor.matmul(
                        pa[:], mt[:, o, :], s2[:, o, :],
                        start=(i == 0), stop=(i == n - 1))
                return pa

            wr_out = make_writer(out_d, 0, NT, f32)
            zacc = {}
            for rpass in range(NREG):
                last = rpass == NREG - 1
                for t in range(NT):
                    cl = [int(bases2[rpass, t]) + j
                          for j in range(int(C2[rpass, t]))]
                    if cl:
                        pa = agg_run(t, cl)
                        if t in zacc:
                            znew = hb_pool.tile([128, 128], f16, tag="zsb",
                                                name="zadd")
                            nc.vector.tensor_tensor(znew[:], pa[:], zacc[t][:],
                                                    op=ALU.add)
                        else:
                            znew = za_pool.tile([128, 128], f16, tag="zA",
                                                name="zA")
                            nc.scalar.activation(znew[:], pa[:], AF.Copy)
                        if not last:
                            if t in zacc:
                                zst = za_pool.tile([128, 128], f16, tag="zA",
                                                   name="zA2")
                                nc.vector.tensor_copy(out=zst[:], in_=znew[:])
                                znew = zst
                        zacc[t] = znew
                    elif last and t not in zacc:
                        znew = za_pool.tile([128, 128], f16, tag="zA", name="zA")
                        nc.vector.memset(znew[:], 0.0)
                        zacc[t] = znew
                    if last:
                        pw2 = psw_pool.tile([128, 128], f32, tag="pw",
                                            name="pw2")
                        nc.tensor.matmul(pw2[:], zacc[t][:], W2f[:])
                        wr_out(t, lambda dst, pw2=pw2, t=t:
                               nc.vector.scalar_tensor_tensor(
                                   dst, pw2[:], ndst_t[:, t:t + 1], B2bc[:],
                                   op0=ALU.mult, op1=ALU.add))

    nc.compile()
    return nc


def kernel(x, src, dst, W1, b1, W2, b2):
    from concourse.bass_utils import run_bass_kernel_spmd

    in_maps, sched, meta = _host_prep(x, src, dst, W1, b1, W2, b2)
    key = (sched, meta)
    if key not in _CACHE:
        _CACHE[key] = _build_program(sched, meta)
    nc = _CACHE[key]
    res = run_bass_kernel_spmd(nc, in_maps, core_ids=list(range(N_CORES)))
    out = np.empty((N_NODES, D), dtype=np.float32)
    for k in range(N_CORES):
        out[k * NPC: (k + 1) * NPC] = res.results[k]["out"]
    return out
